# revision 10
# baseline (speedup 1.0000x reference)
"""Causal single-head attention (B=4, S=2048, D=1024) on 8 TRN2 NeuronCores.

Sharding: 2 cores per batch; each core owns 8 q-blocks of 128 rows chosen so
both cores of a batch see the same multiset of causal kv-span lengths:
core h=0 -> q-blocks [0,3,4,7,8,11,12,15], core h=1 -> [1,2,5,6,9,10,13,14];
padded pair-spans W = 256*(pos+1). One SPMD program serves all 8 cores;
per-core differences (which q rows, causal mask offsets) ride in the data.

Math per core (bf16 operands, fp32 PSUM accumulation), with the host folding
M = Wq @ Wk^T / sqrt(D) so no K-projection is needed on device:
  A^T = M^T @ qT                                      (single projection)
  S_i = A_i^T.T @ kT (+ additive causal mask)         (scores vs RAW k^T)
  P = exp(S), denom = rowsum(P)                       (no max-sub: |S| small)
  T_i = (P @ v) / denom                               (reassociated: raw v)
  out_i = T_i @ Wv                                    (deferred out-proj)

Everything is SBUF-resident in bf16 (no DRAM spill). The whole A-projection
runs first (its matmuls hide the kT/v/wv streaming), then blocks flow in
ascending span order with block i's finish (denominator, normalize, T
transpose, out-projection) emitted after block i+1's attention so the
cross-engine stats chain never stalls the in-order tensor queue.
"""

import os

import ml_dtypes
import numpy as np

import concourse.bass as bass
import concourse.mybir as mybir
import concourse.tile as tile
from concourse import bacc
from concourse.bass_utils import run_bass_kernel_spmd

B, S, D = 4, 2048, 1024
P = 128                      # partitions / q-block rows
NBLK = 8                     # q-blocks per core
CH = 512                     # kv chunk (matmul moving free dim)
KO = D // P                  # 8 contraction chunks
NV = S // P                  # 16 v row-chunks
W = [256, 512, 768, 1024, 1280, 1536, 1792, 2048]   # padded pair spans
BLOCKS = [[0, 3, 4, 7, 8, 11, 12, 15], [1, 2, 5, 6, 9, 10, 13, 14]]
BF = mybir.dt.bfloat16
F32 = mybir.dt.float32
NEG = -1e30

_cached = {}


def _build():
    if "nc" in _cached:
        return _cached["nc"]
    nc = bacc.Bacc("TRN2", target_bir_lowering=False, debug=False, num_devices=8)
    qT = nc.dram_tensor("qT", [D, P * NBLK], BF, kind="ExternalInput").ap()
    kT = nc.dram_tensor("kT", [D, S], BF, kind="ExternalInput").ap()
    v = nc.dram_tensor("v", [S, D], BF, kind="ExternalInput").ap()
    wq = nc.dram_tensor("wq", [D, D], BF, kind="ExternalInput").ap()
    wv = nc.dram_tensor("wv", [D, D], BF, kind="ExternalInput").ap()
    mask = nc.dram_tensor("mask", [P, NBLK, CH], BF, kind="ExternalInput").ap()
    ident = nc.dram_tensor("ident", [P, P], BF, kind="ExternalInput").ap()
    out = nc.dram_tensor("out", [P * NBLK, D], BF, kind="ExternalOutput").ap()

    kT_r = kT.rearrange("(ko p) s -> p ko s", p=P)
    v_r = v.rearrange("(so p) d -> p so d", p=P)
    wv_r = wv.rearrange("(ko p) m -> p ko m", p=P)
    wq_r = wq.rearrange("(ko p) m -> p ko m", p=P)
    qT_r = qT.rearrange("(ko p) s -> p ko s", p=P)

    with tile.TileContext(nc) as tc:
        with tc.tile_pool(name="pers", bufs=1) as pers, \
             tc.tile_pool(name="qw", bufs=1) as qw, \
             tc.tile_pool(name="ppool", bufs=4) as ppool, \
             tc.tile_pool(name="ptpool", bufs=6) as ptpool, \
             tc.tile_pool(name="tpool", bufs=3) as tpool, \
             tc.tile_pool(name="ttpool", bufs=3) as ttpool, \
             tc.tile_pool(name="opool", bufs=3) as opool, \
             tc.tile_pool(name="cwork", bufs=2) as cwork, \
             tc.tile_pool(name="pswork", bufs=2, space="PSUM") as pswork, \
             tc.tile_pool(name="ps_s", bufs=2, space="PSUM") as ps_s, \
             tc.tile_pool(name="ps_t", bufs=1, space="PSUM") as ps_t:

            ident_sb = pers.tile([P, P], BF)
            # preload the scalar-engine Exp table before the hot loop
            warm_in = pers.tile([P, 1], F32)
            nc.vector.memset(warm_in[:], 0.0)
            warm_out = pers.tile([P, 1], BF)
            nc.scalar.activation(warm_out[:], warm_in[:],
                                 mybir.ActivationFunctionType.Exp)

            mask_sb = pers.tile([P, NBLK, CH], BF)
            QT_sb = pers.tile([P, KO, P * NBLK], BF)
            KT_sb = pers.tile([P, KO, S], BF)
            V_sb = pers.tile([P, NV, D], BF)
            WV_sb = pers.tile([P, KO, D], BF)
            qT_sb = qw.tile([P, KO, P * NBLK], BF)
            wq_sb = qw.tile([P, KO, D], BF)

            # ---- DMA emission: first-use order. Descriptor issue costs
            # ~600ns each on the sync engine while the fabric moves
            # ~0.38 MB/us, so the head uses 2-ko (512KB) granules to balance
            # issue rate against transfer progress.
            nc.sync.dma_start(ident_sb[:], ident)
            for g in range(0, KO, 2):
                nc.sync.dma_start(wq_sb[:, g:g + 2, 0:CH],
                                  wq_r[:, g:g + 2, 0:CH])
                nc.sync.dma_start(qT_sb[:, g:g + 2, 0:CH],
                                  qT_r[:, g:g + 2, 0:CH])
            nc.sync.dma_start(wq_sb[:, :, CH:D], wq_r[:, :, CH:D])
            nc.sync.dma_start(qT_sb[:, :, CH:P * NBLK], qT_r[:, :, CH:P * NBLK])
            nc.sync.dma_start(KT_sb[:, :, 0:2 * CH], kT_r[:, :, 0:2 * CH])
            nc.sync.dma_start(mask_sb[:], mask)
            nc.sync.dma_start(V_sb[:, 0:4], v_r[:, 0:4])
            nc.sync.dma_start(WV_sb[:], wv_r[:])
            nc.sync.dma_start(KT_sb[:, :, 2 * CH:S], kT_r[:, :, 2 * CH:S])
            nc.sync.dma_start(V_sb[:, 4:NV], v_r[:, 4:NV])

            # spin the PE on dummy transposes while the head streams in:
            # keeps the HAM clock-gate warm so the first real matmuls run
            # at full rate instead of 1.2 GHz.
            spin_ps = pswork.tile([P, P], BF, tag="w", name="spin_ps")
            for _ in range(28):
                nc.tensor.transpose(spin_ps[:], ident_sb[:], ident_sb[:])
            spin_out = ptpool.tile([P, P], BF, tag="pt", name="spin_out")
            nc.vector.tensor_copy(spin_out[:], spin_ps[:])

            def a_proj():
                for n in range(2):
                    for m in range(KO):
                        ps = pswork.tile([P, CH], F32, tag="w",
                                         name=f"ap_{n}_{m}")
                        for k in range(KO):
                            nc.tensor.matmul(
                                ps[:], wq_sb[:, k, bass.ts(m, P)],
                                qT_sb[:, k, bass.ts(n, CH)],
                                start=(k == 0), stop=(k == KO - 1))
                        nc.vector.tensor_copy(QT_sb[:, m, bass.ts(n, CH)], ps[:])

            def attention_core(i):
                """Scores + exp + P-transpose + AV. Returns finish state."""
                wi = W[i]
                nch = (wi + CH - 1) // CH
                nkv = wi // P
                par = "e" if i % 2 == 0 else "o"
                ps_T0 = ps_t.tile([P, CH], F32, tag=f"T0{par}", name=f"T0_{i}")
                ps_T1 = ps_t.tile([P, CH], F32, tag=f"T1{par}", name=f"T1_{i}")
                dsums = []
                p_tiles = []

                def emit_scores(c):
                    w = min(CH, wi - c * CH)
                    ps_c = ps_s.tile([P, CH], F32, tag="s", name=f"s_{i}_{c}")
                    for k in range(KO):
                        nc.tensor.matmul(
                            ps_c[:, 0:w], QT_sb[:, k, bass.ts(i, P)],
                            KT_sb[:, k, bass.ds(c * CH, w)],
                            start=(k == 0), stop=(k == KO - 1))
                    if c == nch - 1:
                        nc.vector.tensor_tensor(
                            ps_c[:, 0:w], ps_c[:, 0:w],
                            mask_sb[:, i, 0:w], mybir.AluOpType.add)
                    p_sb = ppool.tile([P, CH], BF, tag="p", name=f"p_{i}_{c}")
                    ds_t = cwork.tile([P, 1], F32, tag="ds", bufs=8,
                                      name=f"ds_{i}_{c}")
                    nc.scalar.activation(
                        p_sb[:, 0:w], ps_c[:, 0:w],
                        mybir.ActivationFunctionType.Exp, accum_out=ds_t[:])
                    dsums.append(ds_t)
                    p_tiles.append(p_sb)

                def emit_av(c, t, pt_sb):
                    kvi = c * (CH // P) + t
                    vc = V_sb[:, kvi]
                    nc.tensor.matmul(
                        ps_T0[:], pt_sb[:], vc[:, 0:CH],
                        start=(kvi == 0), stop=(kvi == nkv - 1))
                    nc.tensor.matmul(
                        ps_T1[:], pt_sb[:], vc[:, CH:D],
                        start=(kvi == 0), stop=(kvi == nkv - 1))

                def emit_trav(c):
                    # transposes run 2 ahead of the AV matmuls
                    nt = min(CH, wi - c * CH) // P
                    pts = []
                    for t in range(nt):
                        ptr = pswork.tile([P, P], BF, tag="w",
                                          name=f"ptr_{i}_{c}_{t}")
                        nc.tensor.transpose(
                            ptr[:], p_tiles[c][:, bass.ts(t, P)], ident_sb[:])
                        pt_sb = ptpool.tile([P, P], BF, tag="pt")
                        nc.vector.tensor_copy(pt_sb[:], ptr[:])
                        pts.append(pt_sb)
                        if t >= 2:
                            emit_av(c, t - 2, pts[t - 2])
                    for t in range(max(0, nt - 2), nt):
                        emit_av(c, t, pts[t])

                for c in range(nch):
                    emit_scores(c)
                    if c >= 1:
                        emit_trav(c - 1)
                emit_trav(nch - 1)
                return (i, nch, ps_T0, ps_T1, dsums)

            def finish_block(st):
                i, nch, ps_T0, ps_T1, dsums = st
                denom = cwork.tile([P, 1], F32, tag="den", name=f"den_{i}")
                if nch == 1:
                    nc.vector.tensor_copy(denom[:], dsums[0][:])
                else:
                    nc.vector.tensor_tensor(
                        denom[:], dsums[0][:], dsums[1][:], mybir.AluOpType.add)
                    for c in range(2, nch):
                        nc.vector.tensor_tensor(
                            denom[:], denom[:], dsums[c][:], mybir.AluOpType.add)
                rden = cwork.tile([P, 1], F32, tag="rden", name=f"rden_{i}")
                nc.vector.reciprocal(rden[:], denom[:])

                # normalize + downcast on the scalar engine, then transpose T
                t_sb = tpool.tile([P, D], BF, tag="t", name=f"t_{i}")
                nc.scalar.activation(t_sb[:, 0:CH], ps_T0[:],
                                     mybir.ActivationFunctionType.Copy,
                                     scale=rden[:])
                nc.scalar.activation(t_sb[:, CH:D], ps_T1[:],
                                     mybir.ActivationFunctionType.Copy,
                                     scale=rden[:])
                tt_sb = ttpool.tile([P, KO, P], BF, tag="tt", name=f"tt_{i}")
                for d_ in range(KO):
                    ptr = pswork.tile([P, P], BF, tag="w", name=f"ttr_{i}_{d_}")
                    nc.tensor.transpose(
                        ptr[:], t_sb[:, bass.ts(d_, P)], ident_sb[:])
                    nc.vector.tensor_copy(tt_sb[:, d_], ptr[:])

                o_sb = opool.tile([P, D], BF, tag="o", name=f"o_{i}")
                ps_o0 = pswork.tile([P, CH], F32, tag="w", name=f"o0_{i}")
                for d_ in range(KO):
                    nc.tensor.matmul(
                        ps_o0[:], tt_sb[:, d_], WV_sb[:, d_, 0:CH],
                        start=(d_ == 0), stop=(d_ == KO - 1))
                nc.vector.tensor_copy(o_sb[:, 0:CH], ps_o0[:])
                nc.sync.dma_start(out[bass.ts(i, P), 0:CH], o_sb[:, 0:CH])
                ps_o1 = pswork.tile([P, CH], F32, tag="w", name=f"o1_{i}")
                for d_ in range(KO):
                    nc.tensor.matmul(
                        ps_o1[:], tt_sb[:, d_], WV_sb[:, d_, CH:D],
                        start=(d_ == 0), stop=(d_ == KO - 1))
                nc.vector.tensor_copy(o_sb[:, CH:D], ps_o1[:])
                nc.sync.dma_start(out[bass.ts(i, P), CH:D], o_sb[:, CH:D])

            # ---- emission schedule: A-proj, then pipelined blocks ----
            a_proj()
            prev = None
            for i in range(NBLK):
                st = attention_core(i)
                if prev is not None:
                    finish_block(prev)
                prev = st
            finish_block(prev)

    nc.compile()
    _cached["nc"] = nc
    return nc


LAST_RESULT = None


def kernel(q, k, v, Wq, Wk, Wv, mask):
    global LAST_RESULT
    q = np.asarray(q, dtype=np.float32)
    k = np.asarray(k, dtype=np.float32)
    v = np.asarray(v, dtype=np.float32)
    Wq = np.asarray(Wq, dtype=np.float32)
    Wk = np.asarray(Wk, dtype=np.float32)
    Wv = np.asarray(Wv, dtype=np.float32)

    nc = _build()

    bf = ml_dtypes.bfloat16
    wm = np.ascontiguousarray(
        (Wq.astype(np.float64) @ Wk.astype(np.float64).T
         / np.sqrt(np.float64(D))).astype(bf))
    wv_c = np.ascontiguousarray(Wv.astype(bf))
    ident = np.eye(P, dtype=bf)

    masks = []
    r = np.arange(P)[:, None]
    c = np.arange(CH)[None, :]
    for h in range(2):
        m = np.zeros((P, NBLK, CH), dtype=np.float32)
        for i in range(NBLK):
            j = BLOCKS[h][i]
            q0 = P * j
            nch = (W[i] + CH - 1) // CH
            last_off = CH * (nch - 1)
            w_last = W[i] - last_off
            mi = np.where(last_off + c <= q0 + r, 0.0, NEG)
            mi[:, w_last:] = 0.0
            m[:, i, :] = mi
        masks.append(m.astype(bf))

    in_maps = []
    for core in range(8):
        b, h = core // 2, core % 2
        blocks = BLOCKS[h]
        qTb = q[b].T  # [D, S]
        cols = np.concatenate([np.arange(j * P, (j + 1) * P) for j in blocks])
        in_maps.append({
            "qT": np.ascontiguousarray(qTb[:, cols].astype(bf)),
            "kT": np.ascontiguousarray(k[b].T.astype(bf)),
            "v": np.ascontiguousarray(v[b].astype(bf)),
            "wq": wm, "wv": wv_c,
            "mask": masks[h], "ident": ident,
        })

    res = run_bass_kernel_spmd(nc, in_maps, list(range(8)),
                               trace=bool(os.environ.get("KERNEL_TRACE")))
    LAST_RESULT = res

    out = np.empty((B, S, D), dtype=np.float32)
    for core in range(8):
        b, h = core // 2, core % 2
        oc = np.asarray(res.results[core]["out"], dtype=np.float32)
        for pos, j in enumerate(BLOCKS[h]):
            out[b, j * P:(j + 1) * P, :] = oc[pos * P:(pos + 1) * P, :]
    return out


# revision 11
# speedup vs baseline: 1.1867x; 1.1867x over previous
"""Causal single-head attention (B=4, S=2048, D=1024) on 8 TRN2 NeuronCores.

Sharding: 2 cores per batch; each core owns 8 q-blocks of 128 rows chosen so
both cores of a batch see the same multiset of causal kv-span lengths:
core h=0 -> q-blocks [0,3,4,7,8,11,12,15], core h=1 -> [1,2,5,6,9,10,13,14];
padded pair-spans W = 256*(pos+1). One SPMD program serves all 8 cores;
per-core differences (which q rows, causal mask offsets) ride in the data.

Math per core (bf16 operands, fp32 PSUM accumulation), with the host folding
M = Wq @ Wk^T / sqrt(D) so no K-projection is needed on device:
  A^T = M^T @ qT                                      (single projection)
  S_i = A_i^T.T @ kT (+ additive causal mask)         (scores vs RAW k^T)
  P = exp(S), denom = rowsum(P)                       (no max-sub: |S| small)
  T_i = (P @ v) / denom                               (reassociated: raw v)
  out_i = T_i @ Wv                                    (deferred out-proj)

Everything is SBUF-resident in bf16 (no DRAM spill). The whole A-projection
runs first (its matmuls hide the kT/v/wv streaming), then blocks flow in
ascending span order with block i's finish (denominator, normalize, T
transpose, out-projection) emitted after block i+1's attention so the
cross-engine stats chain never stalls the in-order tensor queue.
"""

import os

import ml_dtypes
import numpy as np

import concourse.bass as bass
import concourse.mybir as mybir
import concourse.tile as tile
from concourse import bacc
from concourse.bass_utils import run_bass_kernel_spmd

B, S, D = 4, 2048, 1024
P = 128                      # partitions / q-block rows
NBLK = 8                     # q-blocks per core
CH = 512                     # kv chunk (matmul moving free dim)
KO = D // P                  # 8 contraction chunks
NV = S // P                  # 16 v row-chunks
W = [256, 512, 768, 1024, 1280, 1536, 1792, 2048]   # padded pair spans
BLOCKS = [[0, 3, 4, 7, 8, 11, 12, 15], [1, 2, 5, 6, 9, 10, 13, 14]]
BF = mybir.dt.bfloat16
F32 = mybir.dt.float32
NEG = -1e30

_cached = {}


def _build():
    if "nc" in _cached:
        return _cached["nc"]
    nc = bacc.Bacc("TRN2", target_bir_lowering=False, debug=False, num_devices=8)
    qT = nc.dram_tensor("qT", [D, P * NBLK], BF, kind="ExternalInput").ap()
    kT = nc.dram_tensor("kT", [D, S], BF, kind="ExternalInput").ap()
    v = nc.dram_tensor("v", [S, D], BF, kind="ExternalInput").ap()
    wq = nc.dram_tensor("wq", [D, D], BF, kind="ExternalInput").ap()
    wv = nc.dram_tensor("wv", [D, D], BF, kind="ExternalInput").ap()
    mask = nc.dram_tensor("mask", [P, NBLK, CH], BF, kind="ExternalInput").ap()
    ident = nc.dram_tensor("ident", [P, P], BF, kind="ExternalInput").ap()
    out = nc.dram_tensor("out", [P * NBLK, D], BF, kind="ExternalOutput").ap()

    kT_r = kT.rearrange("(ko p) s -> p ko s", p=P)
    v_r = v.rearrange("(so p) d -> p so d", p=P)
    wv_r = wv.rearrange("(ko p) m -> p ko m", p=P)
    wq_r = wq.rearrange("(ko p) m -> p ko m", p=P)
    qT_r = qT.rearrange("(ko p) s -> p ko s", p=P)

    with tile.TileContext(nc) as tc:
        with tc.tile_pool(name="pers", bufs=1) as pers, \
             tc.tile_pool(name="qw", bufs=1) as qw, \
             tc.tile_pool(name="ppool", bufs=4) as ppool, \
             tc.tile_pool(name="ptpool", bufs=6) as ptpool, \
             tc.tile_pool(name="tpool", bufs=3) as tpool, \
             tc.tile_pool(name="ttpool", bufs=3) as ttpool, \
             tc.tile_pool(name="opool", bufs=3) as opool, \
             tc.tile_pool(name="cwork", bufs=2) as cwork, \
             tc.tile_pool(name="pswork", bufs=2, space="PSUM") as pswork, \
             tc.tile_pool(name="ps_s", bufs=2, space="PSUM") as ps_s, \
             tc.tile_pool(name="ps_t", bufs=1, space="PSUM") as ps_t:

            ident_sb = pers.tile([P, P], BF)
            # preload the scalar-engine Exp table before the hot loop
            warm_in = pers.tile([P, 1], F32)
            nc.vector.memset(warm_in[:], 0.0)
            warm_out = pers.tile([P, 1], BF)
            nc.scalar.activation(warm_out[:], warm_in[:],
                                 mybir.ActivationFunctionType.Exp)

            mask_sb = pers.tile([P, NBLK, CH], BF)
            QT_sb = pers.tile([P, KO, P * NBLK], BF)
            KT_sb = pers.tile([P, KO, S], BF)
            V_sb = pers.tile([P, NV, D], BF)
            WV_sb = pers.tile([P, KO, D], BF)
            qT_sb = qw.tile([P, KO, P * NBLK], BF)
            wq_sb = qw.tile([P, KO, D], BF)

            # ---- DMA emission: first-use order. Descriptor issue costs
            # ~600ns each on the sync engine while the fabric moves
            # ~0.38 MB/us, so the head uses 2-ko (512KB) granules to balance
            # issue rate against transfer progress.
            nc.sync.dma_start(ident_sb[:], ident)
            for g in range(0, KO, 2):
                nc.sync.dma_start(wq_sb[:, g:g + 2, 0:CH],
                                  wq_r[:, g:g + 2, 0:CH])
                nc.sync.dma_start(qT_sb[:, g:g + 2, 0:CH],
                                  qT_r[:, g:g + 2, 0:CH])
            nc.sync.dma_start(wq_sb[:, :, CH:D], wq_r[:, :, CH:D])
            nc.sync.dma_start(qT_sb[:, :, CH:P * NBLK], qT_r[:, :, CH:P * NBLK])
            nc.sync.dma_start(KT_sb[:, :, 0:2 * CH], kT_r[:, :, 0:2 * CH])
            nc.sync.dma_start(mask_sb[:], mask)
            nc.sync.dma_start(V_sb[:, 0:4], v_r[:, 0:4])
            nc.sync.dma_start(WV_sb[:], wv_r[:])
            nc.sync.dma_start(KT_sb[:, :, 2 * CH:S], kT_r[:, :, 2 * CH:S])
            nc.sync.dma_start(V_sb[:, 4:NV], v_r[:, 4:NV])

            def a_proj():
                for n in range(2):
                    for m in range(KO):
                        ps = pswork.tile([P, CH], F32, tag="w",
                                         name=f"ap_{n}_{m}")
                        for k in range(KO):
                            nc.tensor.matmul(
                                ps[:], wq_sb[:, k, bass.ts(m, P)],
                                qT_sb[:, k, bass.ts(n, CH)],
                                start=(k == 0), stop=(k == KO - 1))
                        nc.vector.tensor_copy(QT_sb[:, m, bass.ts(n, CH)], ps[:])

            def attention_core(i):
                """Scores + exp + P-transpose + AV. Returns finish state."""
                wi = W[i]
                nch = (wi + CH - 1) // CH
                nkv = wi // P
                par = "e" if i % 2 == 0 else "o"
                ps_T0 = ps_t.tile([P, CH], F32, tag=f"T0{par}", name=f"T0_{i}")
                ps_T1 = ps_t.tile([P, CH], F32, tag=f"T1{par}", name=f"T1_{i}")
                dsums = []
                p_tiles = []

                def emit_scores(c):
                    w = min(CH, wi - c * CH)
                    ps_c = ps_s.tile([P, CH], F32, tag="s", name=f"s_{i}_{c}")
                    for k in range(KO):
                        nc.tensor.matmul(
                            ps_c[:, 0:w], QT_sb[:, k, bass.ts(i, P)],
                            KT_sb[:, k, bass.ds(c * CH, w)],
                            start=(k == 0), stop=(k == KO - 1))
                    if c == nch - 1:
                        nc.vector.tensor_tensor(
                            ps_c[:, 0:w], ps_c[:, 0:w],
                            mask_sb[:, i, 0:w], mybir.AluOpType.add)
                    p_sb = ppool.tile([P, CH], BF, tag="p", name=f"p_{i}_{c}")
                    ds_t = cwork.tile([P, 1], F32, tag="ds", bufs=8,
                                      name=f"ds_{i}_{c}")
                    nc.scalar.activation(
                        p_sb[:, 0:w], ps_c[:, 0:w],
                        mybir.ActivationFunctionType.Exp, accum_out=ds_t[:])
                    dsums.append(ds_t)
                    p_tiles.append(p_sb)

                def emit_av(c, t, pt_sb):
                    kvi = c * (CH // P) + t
                    vc = V_sb[:, kvi]
                    nc.tensor.matmul(
                        ps_T0[:], pt_sb[:], vc[:, 0:CH],
                        start=(kvi == 0), stop=(kvi == nkv - 1))
                    nc.tensor.matmul(
                        ps_T1[:], pt_sb[:], vc[:, CH:D],
                        start=(kvi == 0), stop=(kvi == nkv - 1))

                def emit_trav(c):
                    # transposes run 2 ahead of the AV matmuls
                    nt = min(CH, wi - c * CH) // P
                    pts = []
                    for t in range(nt):
                        ptr = pswork.tile([P, P], BF, tag="w",
                                          name=f"ptr_{i}_{c}_{t}")
                        nc.tensor.transpose(
                            ptr[:], p_tiles[c][:, bass.ts(t, P)], ident_sb[:])
                        pt_sb = ptpool.tile([P, P], BF, tag="pt")
                        nc.vector.tensor_copy(pt_sb[:], ptr[:])
                        pts.append(pt_sb)
                        if t >= 2:
                            emit_av(c, t - 2, pts[t - 2])
                    for t in range(max(0, nt - 2), nt):
                        emit_av(c, t, pts[t])

                for c in range(nch):
                    emit_scores(c)
                    if c >= 1:
                        emit_trav(c - 1)
                emit_trav(nch - 1)
                return (i, nch, ps_T0, ps_T1, dsums)

            def finish_block(st):
                i, nch, ps_T0, ps_T1, dsums = st
                denom = cwork.tile([P, 1], F32, tag="den", name=f"den_{i}")
                if nch == 1:
                    nc.vector.tensor_copy(denom[:], dsums[0][:])
                else:
                    nc.vector.tensor_tensor(
                        denom[:], dsums[0][:], dsums[1][:], mybir.AluOpType.add)
                    for c in range(2, nch):
                        nc.vector.tensor_tensor(
                            denom[:], denom[:], dsums[c][:], mybir.AluOpType.add)
                rden = cwork.tile([P, 1], F32, tag="rden", name=f"rden_{i}")
                nc.vector.reciprocal(rden[:], denom[:])

                # normalize + downcast on the scalar engine, then transpose T
                t_sb = tpool.tile([P, D], BF, tag="t", name=f"t_{i}")
                nc.scalar.activation(t_sb[:, 0:CH], ps_T0[:],
                                     mybir.ActivationFunctionType.Copy,
                                     scale=rden[:])
                nc.scalar.activation(t_sb[:, CH:D], ps_T1[:],
                                     mybir.ActivationFunctionType.Copy,
                                     scale=rden[:])
                tt_sb = ttpool.tile([P, KO, P], BF, tag="tt", name=f"tt_{i}")
                for d_ in range(KO):
                    ptr = pswork.tile([P, P], BF, tag="w", name=f"ttr_{i}_{d_}")
                    nc.tensor.transpose(
                        ptr[:], t_sb[:, bass.ts(d_, P)], ident_sb[:])
                    nc.vector.tensor_copy(tt_sb[:, d_], ptr[:])

                o_sb = opool.tile([P, D], BF, tag="o", name=f"o_{i}")
                ps_o0 = pswork.tile([P, CH], F32, tag="w", name=f"o0_{i}")
                for d_ in range(KO):
                    nc.tensor.matmul(
                        ps_o0[:], tt_sb[:, d_], WV_sb[:, d_, 0:CH],
                        start=(d_ == 0), stop=(d_ == KO - 1))
                nc.vector.tensor_copy(o_sb[:, 0:CH], ps_o0[:])
                nc.sync.dma_start(out[bass.ts(i, P), 0:CH], o_sb[:, 0:CH])
                ps_o1 = pswork.tile([P, CH], F32, tag="w", name=f"o1_{i}")
                for d_ in range(KO):
                    nc.tensor.matmul(
                        ps_o1[:], tt_sb[:, d_], WV_sb[:, d_, CH:D],
                        start=(d_ == 0), stop=(d_ == KO - 1))
                nc.vector.tensor_copy(o_sb[:, CH:D], ps_o1[:])
                nc.sync.dma_start(out[bass.ts(i, P), CH:D], o_sb[:, CH:D])

            # ---- emission schedule: A-proj, then pipelined blocks ----
            a_proj()
            prev = None
            for i in range(NBLK):
                st = attention_core(i)
                if prev is not None:
                    finish_block(prev)
                prev = st
            finish_block(prev)

    nc.compile()
    _cached["nc"] = nc
    return nc


LAST_RESULT = None


def kernel(q, k, v, Wq, Wk, Wv, mask):
    global LAST_RESULT
    q = np.asarray(q, dtype=np.float32)
    k = np.asarray(k, dtype=np.float32)
    v = np.asarray(v, dtype=np.float32)
    Wq = np.asarray(Wq, dtype=np.float32)
    Wk = np.asarray(Wk, dtype=np.float32)
    Wv = np.asarray(Wv, dtype=np.float32)

    nc = _build()

    bf = ml_dtypes.bfloat16
    wm = np.ascontiguousarray(
        (Wq.astype(np.float64) @ Wk.astype(np.float64).T
         / np.sqrt(np.float64(D))).astype(bf))
    wv_c = np.ascontiguousarray(Wv.astype(bf))
    ident = np.eye(P, dtype=bf)

    masks = []
    r = np.arange(P)[:, None]
    c = np.arange(CH)[None, :]
    for h in range(2):
        m = np.zeros((P, NBLK, CH), dtype=np.float32)
        for i in range(NBLK):
            j = BLOCKS[h][i]
            q0 = P * j
            nch = (W[i] + CH - 1) // CH
            last_off = CH * (nch - 1)
            w_last = W[i] - last_off
            mi = np.where(last_off + c <= q0 + r, 0.0, NEG)
            mi[:, w_last:] = 0.0
            m[:, i, :] = mi
        masks.append(m.astype(bf))

    in_maps = []
    for core in range(8):
        b, h = core // 2, core % 2
        blocks = BLOCKS[h]
        qTb = q[b].T  # [D, S]
        cols = np.concatenate([np.arange(j * P, (j + 1) * P) for j in blocks])
        in_maps.append({
            "qT": np.ascontiguousarray(qTb[:, cols].astype(bf)),
            "kT": np.ascontiguousarray(k[b].T.astype(bf)),
            "v": np.ascontiguousarray(v[b].astype(bf)),
            "wq": wm, "wv": wv_c,
            "mask": masks[h], "ident": ident,
        })

    res = run_bass_kernel_spmd(nc, in_maps, list(range(8)),
                               trace=bool(os.environ.get("KERNEL_TRACE")))
    LAST_RESULT = res

    out = np.empty((B, S, D), dtype=np.float32)
    for core in range(8):
        b, h = core // 2, core % 2
        oc = np.asarray(res.results[core]["out"], dtype=np.float32)
        for pos, j in enumerate(BLOCKS[h]):
            out[b, j * P:(j + 1) * P, :] = oc[pos * P:(pos + 1) * P, :]
    return out


# revision 12
# speedup vs baseline: 1.1921x; 1.0045x over previous
"""Causal single-head attention (B=4, S=2048, D=1024) on 8 TRN2 NeuronCores.

Sharding: 2 cores per batch; each core owns 8 q-blocks of 128 rows chosen so
both cores of a batch see the same multiset of causal kv-span lengths:
core h=0 -> q-blocks [0,3,4,7,8,11,12,15], core h=1 -> [1,2,5,6,9,10,13,14];
padded pair-spans W = 256*(pos+1). One SPMD program serves all 8 cores;
per-core differences (which q rows, causal mask offsets) ride in the data.

Math per core (bf16 operands, fp32 PSUM accumulation), with the host folding
M = Wq @ Wk^T / sqrt(D) so no K-projection is needed on device:
  A^T = M^T @ qT                                      (single projection)
  S_i = A_i^T.T @ kT (+ additive causal mask)         (scores vs RAW k^T)
  P = exp(S), denom = rowsum(P)                       (no max-sub: |S| small)
  T_i = (P @ v) / denom                               (reassociated: raw v)
  out_i = T_i @ Wv                                    (deferred out-proj)

Everything is SBUF-resident in bf16 (no DRAM spill). The whole A-projection
runs first (its matmuls hide the kT/v/wv streaming), then blocks flow in
ascending span order with block i's finish (denominator, normalize, T
transpose, out-projection) emitted after block i+1's attention so the
cross-engine stats chain never stalls the in-order tensor queue.
"""

import os

import ml_dtypes
import numpy as np

import concourse.bass as bass
import concourse.mybir as mybir
import concourse.tile as tile
from concourse import bacc
from concourse.bass_utils import run_bass_kernel_spmd

B, S, D = 4, 2048, 1024
P = 128                      # partitions / q-block rows
NBLK = 8                     # q-blocks per core
CH = 512                     # kv chunk (matmul moving free dim)
KO = D // P                  # 8 contraction chunks
NV = S // P                  # 16 v row-chunks
W = [256, 512, 768, 1024, 1280, 1536, 1792, 2048]   # padded pair spans
BLOCKS = [[0, 3, 4, 7, 8, 11, 12, 15], [1, 2, 5, 6, 9, 10, 13, 14]]
BF = mybir.dt.bfloat16
F32 = mybir.dt.float32
NEG = -1e30

_cached = {}


def _build():
    if "nc" in _cached:
        return _cached["nc"]
    nc = bacc.Bacc("TRN2", target_bir_lowering=False, debug=False, num_devices=8)
    qT = nc.dram_tensor("qT", [D, P * NBLK], BF, kind="ExternalInput").ap()
    kT = nc.dram_tensor("kT", [D, S], BF, kind="ExternalInput").ap()
    v = nc.dram_tensor("v", [S, D], BF, kind="ExternalInput").ap()
    wq = nc.dram_tensor("wq", [D, D], BF, kind="ExternalInput").ap()
    wv = nc.dram_tensor("wv", [D, D], BF, kind="ExternalInput").ap()
    mask = nc.dram_tensor("mask", [P, NBLK, CH], BF, kind="ExternalInput").ap()
    ident = nc.dram_tensor("ident", [P, P], BF, kind="ExternalInput").ap()
    out = nc.dram_tensor("out", [P * NBLK, D], BF, kind="ExternalOutput").ap()

    kT_r = kT.rearrange("(ko p) s -> p ko s", p=P)
    v_r = v.rearrange("(so p) d -> p so d", p=P)
    wv_r = wv.rearrange("(ko p) m -> p ko m", p=P)
    wq_r = wq.rearrange("(ko p) m -> p ko m", p=P)
    qT_r = qT.rearrange("(ko p) s -> p ko s", p=P)

    with tile.TileContext(nc) as tc:
        with tc.tile_pool(name="pers", bufs=1) as pers, \
             tc.tile_pool(name="qw", bufs=1) as qw, \
             tc.tile_pool(name="ppool", bufs=4) as ppool, \
             tc.tile_pool(name="ptpool", bufs=6) as ptpool, \
             tc.tile_pool(name="tpool", bufs=3) as tpool, \
             tc.tile_pool(name="ttpool", bufs=3) as ttpool, \
             tc.tile_pool(name="opool", bufs=3) as opool, \
             tc.tile_pool(name="cwork", bufs=2) as cwork, \
             tc.tile_pool(name="pswork", bufs=2, space="PSUM") as pswork, \
             tc.tile_pool(name="ps_s", bufs=2, space="PSUM") as ps_s, \
             tc.tile_pool(name="ps_t", bufs=1, space="PSUM") as ps_t:

            ident_sb = pers.tile([P, P], BF)
            # preload the scalar-engine Exp table before the hot loop
            warm_in = pers.tile([P, 1], F32)
            nc.vector.memset(warm_in[:], 0.0)
            warm_out = pers.tile([P, 1], BF)
            nc.scalar.activation(warm_out[:], warm_in[:],
                                 mybir.ActivationFunctionType.Exp)

            mask_sb = pers.tile([P, NBLK, CH], BF)
            QT_sb = pers.tile([P, KO, P * NBLK], BF)
            KT_sb = pers.tile([P, KO, S], BF)
            V_sb = pers.tile([P, NV, D], BF)
            WV_sb = pers.tile([P, KO, D], BF)
            qT_sb = qw.tile([P, KO, P * NBLK], BF)
            wq_sb = qw.tile([P, KO, D], BF)

            # ---- DMA emission: first-use order. Descriptor issue costs
            # ~600ns each on the sync engine while the fabric moves
            # ~0.38 MB/us, so the head uses 2-ko (512KB) granules to balance
            # issue rate against transfer progress.
            nc.sync.dma_start(ident_sb[:], ident)
            for g in range(0, KO, 2):
                nc.sync.dma_start(wq_sb[:, g:g + 2, 0:CH],
                                  wq_r[:, g:g + 2, 0:CH])
                nc.sync.dma_start(qT_sb[:, g:g + 2, 0:CH],
                                  qT_r[:, g:g + 2, 0:CH])
            nc.sync.dma_start(wq_sb[:, :, CH:D], wq_r[:, :, CH:D])
            nc.sync.dma_start(qT_sb[:, :, CH:P * NBLK], qT_r[:, :, CH:P * NBLK])
            nc.sync.dma_start(KT_sb[:, :, 0:2 * CH], kT_r[:, :, 0:2 * CH])
            nc.sync.dma_start(mask_sb[:], mask)
            nc.sync.dma_start(V_sb[:, 0:4], v_r[:, 0:4])
            nc.sync.dma_start(WV_sb[:], wv_r[:])
            nc.sync.dma_start(KT_sb[:, :, 2 * CH:S], kT_r[:, :, 2 * CH:S])
            nc.sync.dma_start(V_sb[:, 4:NV], v_r[:, 4:NV])

            # Warm the PE clock-gate while the head DMAs stream: one psum
            # accumulation group of back-to-back ident matmuls (no
            # per-instruction semaphore waits inside a group).
            NSPIN = 20
            spin_ps = pswork.tile([P, P], F32, tag="w", name="spin_ps")
            for si in range(NSPIN):
                nc.tensor.matmul(spin_ps[:], ident_sb[:], ident_sb[:],
                                 start=(si == 0), stop=(si == NSPIN - 1))
            spin_out = ptpool.tile([P, P], BF, tag="pt", name="spin_out")
            nc.vector.tensor_copy(spin_out[:], spin_ps[:])

            def a_proj():
                for n in range(2):
                    for m in range(KO):
                        ps = pswork.tile([P, CH], F32, tag="w",
                                         name=f"ap_{n}_{m}")
                        for k in range(KO):
                            nc.tensor.matmul(
                                ps[:], wq_sb[:, k, bass.ts(m, P)],
                                qT_sb[:, k, bass.ts(n, CH)],
                                start=(k == 0), stop=(k == KO - 1))
                        nc.vector.tensor_copy(QT_sb[:, m, bass.ts(n, CH)], ps[:])

            def attention_core(i):
                """Scores + exp + P-transpose + AV. Returns finish state."""
                wi = W[i]
                nch = (wi + CH - 1) // CH
                nkv = wi // P
                par = "e" if i % 2 == 0 else "o"
                ps_T0 = ps_t.tile([P, CH], F32, tag=f"T0{par}", name=f"T0_{i}")
                ps_T1 = ps_t.tile([P, CH], F32, tag=f"T1{par}", name=f"T1_{i}")
                dsums = []
                p_tiles = []

                def emit_scores(c):
                    w = min(CH, wi - c * CH)
                    ps_c = ps_s.tile([P, CH], F32, tag="s", name=f"s_{i}_{c}")
                    for k in range(KO):
                        nc.tensor.matmul(
                            ps_c[:, 0:w], QT_sb[:, k, bass.ts(i, P)],
                            KT_sb[:, k, bass.ds(c * CH, w)],
                            start=(k == 0), stop=(k == KO - 1))
                    if c == nch - 1:
                        nc.vector.tensor_tensor(
                            ps_c[:, 0:w], ps_c[:, 0:w],
                            mask_sb[:, i, 0:w], mybir.AluOpType.add)
                    p_sb = ppool.tile([P, CH], BF, tag="p", name=f"p_{i}_{c}")
                    ds_t = cwork.tile([P, 1], F32, tag="ds", bufs=8,
                                      name=f"ds_{i}_{c}")
                    nc.scalar.activation(
                        p_sb[:, 0:w], ps_c[:, 0:w],
                        mybir.ActivationFunctionType.Exp, accum_out=ds_t[:])
                    dsums.append(ds_t)
                    p_tiles.append(p_sb)

                def emit_av(c, t, pt_sb):
                    kvi = c * (CH // P) + t
                    vc = V_sb[:, kvi]
                    nc.tensor.matmul(
                        ps_T0[:], pt_sb[:], vc[:, 0:CH],
                        start=(kvi == 0), stop=(kvi == nkv - 1))
                    nc.tensor.matmul(
                        ps_T1[:], pt_sb[:], vc[:, CH:D],
                        start=(kvi == 0), stop=(kvi == nkv - 1))

                def emit_trav(c):
                    # transposes run 2 ahead of the AV matmuls
                    nt = min(CH, wi - c * CH) // P
                    pts = []
                    for t in range(nt):
                        ptr = pswork.tile([P, P], BF, tag="w",
                                          name=f"ptr_{i}_{c}_{t}")
                        nc.tensor.transpose(
                            ptr[:], p_tiles[c][:, bass.ts(t, P)], ident_sb[:])
                        pt_sb = ptpool.tile([P, P], BF, tag="pt")
                        nc.vector.tensor_copy(pt_sb[:], ptr[:])
                        pts.append(pt_sb)
                        if t >= 2:
                            emit_av(c, t - 2, pts[t - 2])
                    for t in range(max(0, nt - 2), nt):
                        emit_av(c, t, pts[t])

                for c in range(nch):
                    emit_scores(c)
                    if c >= 1:
                        emit_trav(c - 1)
                emit_trav(nch - 1)
                return (i, nch, ps_T0, ps_T1, dsums)

            def finish_block(st):
                i, nch, ps_T0, ps_T1, dsums = st
                denom = cwork.tile([P, 1], F32, tag="den", name=f"den_{i}")
                if nch == 1:
                    nc.vector.tensor_copy(denom[:], dsums[0][:])
                else:
                    nc.vector.tensor_tensor(
                        denom[:], dsums[0][:], dsums[1][:], mybir.AluOpType.add)
                    for c in range(2, nch):
                        nc.vector.tensor_tensor(
                            denom[:], denom[:], dsums[c][:], mybir.AluOpType.add)
                rden = cwork.tile([P, 1], F32, tag="rden", name=f"rden_{i}")
                nc.vector.reciprocal(rden[:], denom[:])

                # normalize + downcast on the scalar engine, then transpose T
                t_sb = tpool.tile([P, D], BF, tag="t", name=f"t_{i}")
                nc.scalar.activation(t_sb[:, 0:CH], ps_T0[:],
                                     mybir.ActivationFunctionType.Copy,
                                     scale=rden[:])
                nc.scalar.activation(t_sb[:, CH:D], ps_T1[:],
                                     mybir.ActivationFunctionType.Copy,
                                     scale=rden[:])
                tt_sb = ttpool.tile([P, KO, P], BF, tag="tt", name=f"tt_{i}")
                for d_ in range(KO):
                    ptr = pswork.tile([P, P], BF, tag="w", name=f"ttr_{i}_{d_}")
                    nc.tensor.transpose(
                        ptr[:], t_sb[:, bass.ts(d_, P)], ident_sb[:])
                    nc.vector.tensor_copy(tt_sb[:, d_], ptr[:])

                o_sb = opool.tile([P, D], BF, tag="o", name=f"o_{i}")
                ps_o0 = pswork.tile([P, CH], F32, tag="w", name=f"o0_{i}")
                for d_ in range(KO):
                    nc.tensor.matmul(
                        ps_o0[:], tt_sb[:, d_], WV_sb[:, d_, 0:CH],
                        start=(d_ == 0), stop=(d_ == KO - 1))
                nc.vector.tensor_copy(o_sb[:, 0:CH], ps_o0[:])
                nc.sync.dma_start(out[bass.ts(i, P), 0:CH], o_sb[:, 0:CH])
                ps_o1 = pswork.tile([P, CH], F32, tag="w", name=f"o1_{i}")
                for d_ in range(KO):
                    nc.tensor.matmul(
                        ps_o1[:], tt_sb[:, d_], WV_sb[:, d_, CH:D],
                        start=(d_ == 0), stop=(d_ == KO - 1))
                nc.vector.tensor_copy(o_sb[:, CH:D], ps_o1[:])
                nc.sync.dma_start(out[bass.ts(i, P), CH:D], o_sb[:, CH:D])

            # ---- emission schedule: A-proj, then pipelined blocks ----
            a_proj()
            prev = None
            for i in range(NBLK):
                st = attention_core(i)
                if prev is not None:
                    finish_block(prev)
                prev = st
            finish_block(prev)

    nc.compile()
    _cached["nc"] = nc
    return nc


LAST_RESULT = None


def kernel(q, k, v, Wq, Wk, Wv, mask):
    global LAST_RESULT
    q = np.asarray(q, dtype=np.float32)
    k = np.asarray(k, dtype=np.float32)
    v = np.asarray(v, dtype=np.float32)
    Wq = np.asarray(Wq, dtype=np.float32)
    Wk = np.asarray(Wk, dtype=np.float32)
    Wv = np.asarray(Wv, dtype=np.float32)

    nc = _build()

    bf = ml_dtypes.bfloat16
    wm = np.ascontiguousarray(
        (Wq.astype(np.float64) @ Wk.astype(np.float64).T
         / np.sqrt(np.float64(D))).astype(bf))
    wv_c = np.ascontiguousarray(Wv.astype(bf))
    ident = np.eye(P, dtype=bf)

    masks = []
    r = np.arange(P)[:, None]
    c = np.arange(CH)[None, :]
    for h in range(2):
        m = np.zeros((P, NBLK, CH), dtype=np.float32)
        for i in range(NBLK):
            j = BLOCKS[h][i]
            q0 = P * j
            nch = (W[i] + CH - 1) // CH
            last_off = CH * (nch - 1)
            w_last = W[i] - last_off
            mi = np.where(last_off + c <= q0 + r, 0.0, NEG)
            mi[:, w_last:] = 0.0
            m[:, i, :] = mi
        masks.append(m.astype(bf))

    in_maps = []
    for core in range(8):
        b, h = core // 2, core % 2
        blocks = BLOCKS[h]
        qTb = q[b].T  # [D, S]
        cols = np.concatenate([np.arange(j * P, (j + 1) * P) for j in blocks])
        in_maps.append({
            "qT": np.ascontiguousarray(qTb[:, cols].astype(bf)),
            "kT": np.ascontiguousarray(k[b].T.astype(bf)),
            "v": np.ascontiguousarray(v[b].astype(bf)),
            "wq": wm, "wv": wv_c,
            "mask": masks[h], "ident": ident,
        })

    res = run_bass_kernel_spmd(nc, in_maps, list(range(8)),
                               trace=bool(os.environ.get("KERNEL_TRACE")))
    LAST_RESULT = res

    out = np.empty((B, S, D), dtype=np.float32)
    for core in range(8):
        b, h = core // 2, core % 2
        oc = np.asarray(res.results[core]["out"], dtype=np.float32)
        for pos, j in enumerate(BLOCKS[h]):
            out[b, j * P:(j + 1) * P, :] = oc[pos * P:(pos + 1) * P, :]
    return out


# revision 18
# speedup vs baseline: 1.1932x; 1.0010x over previous
"""Causal single-head attention (B=4, S=2048, D=1024) on 8 TRN2 NeuronCores.

Sharding: 2 cores per batch; each core owns 8 q-blocks of 128 rows chosen so
both cores of a batch see the same multiset of causal kv-span lengths:
core h=0 -> q-blocks [0,3,4,7,8,11,12,15], core h=1 -> [1,2,5,6,9,10,13,14];
padded pair-spans W = 256*(pos+1). One SPMD program serves all 8 cores;
per-core differences (which q rows, causal mask offsets) ride in the data.

Math per core (bf16 operands, fp32 PSUM accumulation), with the host folding
M = Wq @ Wk^T / sqrt(D) so no K-projection is needed on device:
  A^T = M^T @ qT                                      (single projection)
  S_i = A_i^T.T @ kT (+ additive causal mask)         (scores vs RAW k^T)
  P = exp(S), denom = rowsum(P)                       (no max-sub: |S| small)
  T_i = (P @ v) / denom                               (reassociated: raw v)
  out_i = T_i @ Wv                                    (deferred out-proj)

Everything is SBUF-resident in bf16 (no DRAM spill). The whole A-projection
runs first (its matmuls hide the kT/v/wv streaming), then blocks flow in
ascending span order with block i's finish (denominator, normalize, T
transpose, out-projection) emitted after block i+1's attention so the
cross-engine stats chain never stalls the in-order tensor queue.
"""

import os

import ml_dtypes
import numpy as np

import concourse.bass as bass
import concourse.mybir as mybir
import concourse.tile as tile
from concourse import bacc
from concourse.bass_utils import run_bass_kernel_spmd

B, S, D = 4, 2048, 1024
P = 128                      # partitions / q-block rows
NBLK = 8                     # q-blocks per core
CH = 512                     # kv chunk (matmul moving free dim)
KO = D // P                  # 8 contraction chunks
NV = S // P                  # 16 v row-chunks
W = [256, 512, 768, 1024, 1280, 1536, 1792, 2048]   # padded pair spans
BLOCKS = [[0, 3, 4, 7, 8, 11, 12, 15], [1, 2, 5, 6, 9, 10, 13, 14]]
BF = mybir.dt.bfloat16
F32 = mybir.dt.float32
NEG = -1e30

_cached = {}


def _build():
    if "nc" in _cached:
        return _cached["nc"]
    nc = bacc.Bacc("TRN2", target_bir_lowering=False, debug=False, num_devices=8)
    qT = nc.dram_tensor("qT", [D, P * NBLK], BF, kind="ExternalInput").ap()
    kT = nc.dram_tensor("kT", [D, S], BF, kind="ExternalInput").ap()
    v = nc.dram_tensor("v", [S, D], BF, kind="ExternalInput").ap()
    wq = nc.dram_tensor("wq", [D, D], BF, kind="ExternalInput").ap()
    wv = nc.dram_tensor("wv", [D, D], BF, kind="ExternalInput").ap()
    mask = nc.dram_tensor("mask", [P, NBLK, CH], BF, kind="ExternalInput").ap()
    out = nc.dram_tensor("out", [P * NBLK, D], BF, kind="ExternalOutput").ap()

    kT_r = kT.rearrange("(ko p) s -> p ko s", p=P)
    v_r = v.rearrange("(so p) d -> p so d", p=P)
    wv_r = wv.rearrange("(ko p) m -> p ko m", p=P)
    wq_r = wq.rearrange("(ko p) m -> p ko m", p=P)
    qT_r = qT.rearrange("(ko p) s -> p ko s", p=P)

    with tile.TileContext(nc) as tc:
        with tc.tile_pool(name="pers", bufs=1) as pers, \
             tc.tile_pool(name="qw", bufs=1) as qw, \
             tc.tile_pool(name="ppool", bufs=4) as ppool, \
             tc.tile_pool(name="ptpool", bufs=6) as ptpool, \
             tc.tile_pool(name="tpool", bufs=3) as tpool, \
             tc.tile_pool(name="ttpool", bufs=3) as ttpool, \
             tc.tile_pool(name="opool", bufs=3) as opool, \
             tc.tile_pool(name="cwork", bufs=2) as cwork, \
             tc.tile_pool(name="pswork", bufs=2, space="PSUM") as pswork, \
             tc.tile_pool(name="ps_s", bufs=2, space="PSUM") as ps_s, \
             tc.tile_pool(name="ps_t", bufs=1, space="PSUM") as ps_t:

            # identity built on-device (memset + affine diag select): no DMA
            ident_sb = pers.tile([P, P], BF)
            ones_sb = pers.tile([P, P], BF)
            nc.vector.memset(ones_sb[:], 1.0)
            nc.gpsimd.affine_select(ident_sb[:], ones_sb[:],
                                    pattern=[[-1, P]],
                                    compare_op=mybir.AluOpType.is_equal,
                                    fill=0.0, base=0, channel_multiplier=1)
            # preload the scalar-engine Exp table before the hot loop
            warm_in = pers.tile([P, 1], F32)
            nc.vector.memset(warm_in[:], 0.0)
            warm_out = pers.tile([P, 1], BF)
            nc.scalar.activation(warm_out[:], warm_in[:],
                                 mybir.ActivationFunctionType.Exp)

            mask_sb = pers.tile([P, NBLK, CH], BF)
            QT_sb = pers.tile([P, KO, P * NBLK], BF)
            KT_sb = pers.tile([P, KO, S], BF)
            V_sb = pers.tile([P, NV, D], BF)
            WV_sb = pers.tile([P, KO, D], BF)
            qT_sb = qw.tile([P, KO, P * NBLK], BF)
            wq_sb = qw.tile([P, KO, D], BF)

            # ---- DMA emission: first-use order. Descriptor issue costs
            # ~600ns each on the sync engine while the fabric moves
            # ~0.38 MB/us, so the head uses 2-ko (512KB) granules to balance
            # issue rate against transfer progress.
            for g in range(0, KO, 2):
                nc.sync.dma_start(wq_sb[:, g:g + 2, 0:CH],
                                  wq_r[:, g:g + 2, 0:CH])
                nc.sync.dma_start(qT_sb[:, g:g + 2, 0:CH],
                                  qT_r[:, g:g + 2, 0:CH])
            nc.sync.dma_start(wq_sb[:, :, CH:D], wq_r[:, :, CH:D])
            nc.sync.dma_start(qT_sb[:, :, CH:P * NBLK], qT_r[:, :, CH:P * NBLK])
            nc.sync.dma_start(KT_sb[:, :, 0:2 * CH], kT_r[:, :, 0:2 * CH])
            nc.sync.dma_start(mask_sb[:], mask)
            nc.sync.dma_start(V_sb[:, 0:4], v_r[:, 0:4])
            nc.sync.dma_start(WV_sb[:], wv_r[:])
            nc.sync.dma_start(KT_sb[:, :, 2 * CH:S], kT_r[:, :, 2 * CH:S])
            nc.sync.dma_start(V_sb[:, 4:NV], v_r[:, 4:NV])

            # Warm the PE clock-gate while the head DMAs stream: one psum
            # accumulation group of back-to-back ident matmuls (no
            # per-instruction semaphore waits inside a group).
            NSPIN = 10
            spin_ps = pswork.tile([P, P], F32, tag="w", name="spin_ps")
            for si in range(NSPIN):
                nc.tensor.matmul(spin_ps[:], ident_sb[:], ident_sb[:],
                                 start=(si == 0), stop=(si == NSPIN - 1))
            spin_out = ptpool.tile([P, P], BF, tag="pt", name="spin_out")
            nc.vector.tensor_copy(spin_out[:], spin_ps[:])

            def a_proj():
                for n in range(2):
                    for m in range(KO):
                        ps = pswork.tile([P, CH], F32, tag="w",
                                         name=f"ap_{n}_{m}")
                        for k in range(KO):
                            nc.tensor.matmul(
                                ps[:], wq_sb[:, k, bass.ts(m, P)],
                                qT_sb[:, k, bass.ts(n, CH)],
                                start=(k == 0), stop=(k == KO - 1))
                        nc.vector.tensor_copy(QT_sb[:, m, bass.ts(n, CH)], ps[:])

            def attention_core(i):
                """Scores + exp + P-transpose + AV. Returns finish state."""
                wi = W[i]
                nch = (wi + CH - 1) // CH
                nkv = wi // P
                par = "e" if i % 2 == 0 else "o"
                ps_T0 = ps_t.tile([P, CH], F32, tag=f"T0{par}", name=f"T0_{i}")
                ps_T1 = ps_t.tile([P, CH], F32, tag=f"T1{par}", name=f"T1_{i}")
                dsums = []
                p_tiles = []

                def emit_scores(c):
                    w = min(CH, wi - c * CH)
                    ps_c = ps_s.tile([P, CH], F32, tag="s", name=f"s_{i}_{c}")
                    for k in range(KO):
                        nc.tensor.matmul(
                            ps_c[:, 0:w], QT_sb[:, k, bass.ts(i, P)],
                            KT_sb[:, k, bass.ds(c * CH, w)],
                            start=(k == 0), stop=(k == KO - 1))
                    if c == nch - 1:
                        nc.vector.tensor_tensor(
                            ps_c[:, 0:w], ps_c[:, 0:w],
                            mask_sb[:, i, 0:w], mybir.AluOpType.add)
                    p_sb = ppool.tile([P, CH], BF, tag="p", name=f"p_{i}_{c}")
                    ds_t = cwork.tile([P, 1], F32, tag="ds", bufs=8,
                                      name=f"ds_{i}_{c}")
                    nc.scalar.activation(
                        p_sb[:, 0:w], ps_c[:, 0:w],
                        mybir.ActivationFunctionType.Exp, accum_out=ds_t[:])
                    dsums.append(ds_t)
                    p_tiles.append(p_sb)

                def emit_av(c, t, pt_sb):
                    kvi = c * (CH // P) + t
                    vc = V_sb[:, kvi]
                    nc.tensor.matmul(
                        ps_T0[:], pt_sb[:], vc[:, 0:CH],
                        start=(kvi == 0), stop=(kvi == nkv - 1))
                    nc.tensor.matmul(
                        ps_T1[:], pt_sb[:], vc[:, CH:D],
                        start=(kvi == 0), stop=(kvi == nkv - 1))

                def emit_trav(c):
                    # transposes run 2 ahead of the AV matmuls
                    nt = min(CH, wi - c * CH) // P
                    pts = []
                    for t in range(nt):
                        ptr = pswork.tile([P, P], BF, tag="w",
                                          name=f"ptr_{i}_{c}_{t}")
                        nc.tensor.transpose(
                            ptr[:], p_tiles[c][:, bass.ts(t, P)], ident_sb[:])
                        pt_sb = ptpool.tile([P, P], BF, tag="pt")
                        nc.vector.tensor_copy(pt_sb[:], ptr[:])
                        pts.append(pt_sb)
                        if t >= 2:
                            emit_av(c, t - 2, pts[t - 2])
                    for t in range(max(0, nt - 2), nt):
                        emit_av(c, t, pts[t])

                for c in range(nch):
                    emit_scores(c)
                    if c >= 1:
                        emit_trav(c - 1)
                emit_trav(nch - 1)
                return (i, nch, ps_T0, ps_T1, dsums)

            def finish_block(st):
                i, nch, ps_T0, ps_T1, dsums = st
                denom = cwork.tile([P, 1], F32, tag="den", name=f"den_{i}")
                if nch == 1:
                    nc.vector.tensor_copy(denom[:], dsums[0][:])
                else:
                    nc.vector.tensor_tensor(
                        denom[:], dsums[0][:], dsums[1][:], mybir.AluOpType.add)
                    for c in range(2, nch):
                        nc.vector.tensor_tensor(
                            denom[:], denom[:], dsums[c][:], mybir.AluOpType.add)
                rden = cwork.tile([P, 1], F32, tag="rden", name=f"rden_{i}")
                nc.vector.reciprocal(rden[:], denom[:])

                # normalize + downcast on the scalar engine, then transpose T
                t_sb = tpool.tile([P, D], BF, tag="t", name=f"t_{i}")
                nc.scalar.activation(t_sb[:, 0:CH], ps_T0[:],
                                     mybir.ActivationFunctionType.Copy,
                                     scale=rden[:])
                nc.scalar.activation(t_sb[:, CH:D], ps_T1[:],
                                     mybir.ActivationFunctionType.Copy,
                                     scale=rden[:])
                tt_sb = ttpool.tile([P, KO, P], BF, tag="tt", name=f"tt_{i}")
                for d_ in range(KO):
                    ptr = pswork.tile([P, P], BF, tag="w", name=f"ttr_{i}_{d_}")
                    nc.tensor.transpose(
                        ptr[:], t_sb[:, bass.ts(d_, P)], ident_sb[:])
                    nc.vector.tensor_copy(tt_sb[:, d_], ptr[:])

                o_sb = opool.tile([P, D], BF, tag="o", name=f"o_{i}")
                ps_o0 = pswork.tile([P, CH], F32, tag="w", name=f"o0_{i}")
                for d_ in range(KO):
                    nc.tensor.matmul(
                        ps_o0[:], tt_sb[:, d_], WV_sb[:, d_, 0:CH],
                        start=(d_ == 0), stop=(d_ == KO - 1))
                nc.vector.tensor_copy(o_sb[:, 0:CH], ps_o0[:])
                nc.sync.dma_start(out[bass.ts(i, P), 0:CH], o_sb[:, 0:CH])
                ps_o1 = pswork.tile([P, CH], F32, tag="w", name=f"o1_{i}")
                for d_ in range(KO):
                    nc.tensor.matmul(
                        ps_o1[:], tt_sb[:, d_], WV_sb[:, d_, CH:D],
                        start=(d_ == 0), stop=(d_ == KO - 1))
                nc.vector.tensor_copy(o_sb[:, CH:D], ps_o1[:])
                nc.sync.dma_start(out[bass.ts(i, P), CH:D], o_sb[:, CH:D])

            # ---- emission schedule: A-proj, then pipelined blocks ----
            a_proj()
            prev = None
            for i in range(NBLK):
                st = attention_core(i)
                if prev is not None:
                    finish_block(prev)
                prev = st
            finish_block(prev)

    nc.compile()
    _cached["nc"] = nc
    return nc


LAST_RESULT = None


def kernel(q, k, v, Wq, Wk, Wv, mask):
    global LAST_RESULT
    q = np.asarray(q, dtype=np.float32)
    k = np.asarray(k, dtype=np.float32)
    v = np.asarray(v, dtype=np.float32)
    Wq = np.asarray(Wq, dtype=np.float32)
    Wk = np.asarray(Wk, dtype=np.float32)
    Wv = np.asarray(Wv, dtype=np.float32)

    nc = _build()

    bf = ml_dtypes.bfloat16
    wm = np.ascontiguousarray(
        (Wq.astype(np.float64) @ Wk.astype(np.float64).T
         / np.sqrt(np.float64(D))).astype(bf))
    wv_c = np.ascontiguousarray(Wv.astype(bf))

    masks = []
    r = np.arange(P)[:, None]
    c = np.arange(CH)[None, :]
    for h in range(2):
        m = np.zeros((P, NBLK, CH), dtype=np.float32)
        for i in range(NBLK):
            j = BLOCKS[h][i]
            q0 = P * j
            nch = (W[i] + CH - 1) // CH
            last_off = CH * (nch - 1)
            w_last = W[i] - last_off
            mi = np.where(last_off + c <= q0 + r, 0.0, NEG)
            mi[:, w_last:] = 0.0
            m[:, i, :] = mi
        masks.append(m.astype(bf))

    in_maps = []
    for core in range(8):
        b, h = core // 2, core % 2
        blocks = BLOCKS[h]
        qTb = q[b].T  # [D, S]
        cols = np.concatenate([np.arange(j * P, (j + 1) * P) for j in blocks])
        in_maps.append({
            "qT": np.ascontiguousarray(qTb[:, cols].astype(bf)),
            "kT": np.ascontiguousarray(k[b].T.astype(bf)),
            "v": np.ascontiguousarray(v[b].astype(bf)),
            "wq": wm, "wv": wv_c,
            "mask": masks[h],
        })

    res = run_bass_kernel_spmd(nc, in_maps, list(range(8)),
                               trace=bool(os.environ.get("KERNEL_TRACE")))
    LAST_RESULT = res

    out = np.empty((B, S, D), dtype=np.float32)
    for core in range(8):
        b, h = core // 2, core % 2
        oc = np.asarray(res.results[core]["out"], dtype=np.float32)
        for pos, j in enumerate(BLOCKS[h]):
            out[b, j * P:(j + 1) * P, :] = oc[pos * P:(pos + 1) * P, :]
    return out
